# revision 13
# baseline (speedup 1.0000x reference)
"""Trainium2 Bass kernel for nn_MappingNetwork (histogram_binning).

reference: seeds = searchsorted(linspace(-1e5, 1e5, 1e8, f32), z[:, 0], 'left');
           out = broadcast(seeds[:, None], (16384, 512)).astype(int32)

The buckets are uniform with spacing 2e5/(1e8-1) ~= 0.002, so the searchsorted
index admits the closed-form affine  seed = (z + 1e5) * 500  (the spec's
sharding_hint itself suggests eliminating the bucket buffer via this closed
form).  Computed in f32 it sits within 6 indices of the bit-exact searchsorted
result — validated against an exact f64-fma emulation of the XLA-CPU linspace
over the full data range.  Relative error ~1.2e-7 against seeds of magnitude
5e7, five orders of magnitude inside the 2e-2 gate.

Per core (2048 rows, row r = p*16 + n -> zv[p, n]):
  1. the Pool engine loads the z[:, 0] shard (8KB) into [128, 16] SBUF via
     its software-DGE queue, waits on its own DMA-completion semaphore, and
     computes the seeds itself: one Pool-engine tensor_scalar producing
     width-2 seed pairs (signalled to sync/scalar immediately) and a second
     producing width-10 runs for its own slice — a single-engine
     load->compute chain with no cross-engine DMA-semaphore latency,
  2. all three DMA-capable engines INDEPENDENTLY stage their seed runs
     (16KB / 64KB) to their own internal DRAM scratch and wait on their own
     completion semaphores,
  3. three concurrent DRAM->DRAM broadcast DMAs fan each row's seed run out
     into the 4MB output shard (sync: columns 0-221, scalar: 222-441, Pool:
     442-511 — the Pool slice maximized under the software-DGE
     16K-descriptor ring limit at repeat count 7): src keeps the stride-0
     repeat dim in the middle with a contiguous innermost run (DGE-legal),
     dst walks [row, repeat, run] with every ISA dim count under 2^16.
Every engine holds the end-of-block barrier on its own DMA-completion
semaphores, so the kernel never signals done before the output is in DRAM.

Sharding: batch 16384 -> 8 cores x 2048 rows.
"""

import numpy as np

N_CORES = 8
B = 16384
W = 512
ROWS = B // N_CORES  # 2048 rows per core
P = 128
NQ = ROWS // P  # 16 queries per partition
CW = 2  # sync/scalar staged run width (seed pair)
PW = 10  # Pool staged run width
POOL_COLS = 70  # Pool d2d slice: repeat count 7 -> 2048*7 descs < 16384
HALF = (W - POOL_COLS) // 2  # 221 -> use 222/220 split below
SP_COLS = 222
ACT_COLS = W - POOL_COLS - SP_COLS  # 220

# (engine, first column, n columns, run width)
SPLITS = [
    ("sync", 0, SP_COLS, CW),
    ("scalar", SP_COLS, ACT_COLS, CW),
    ("gpsimd", SP_COLS + ACT_COLS, POOL_COLS, PW),
]

_nc_cache = {}


def build_nc():
    if "nc" in _nc_cache:
        return _nc_cache["nc"]
    import concourse.bass as bass
    import concourse.mybir as mybir

    dt = mybir.dt
    alu = mybir.AluOpType

    nc = bass.Bass(detect_race_conditions=False)
    zcol = nc.dram_tensor("zcol", [ROWS, 1], dt.float32, kind="ExternalInput")
    out = nc.dram_tensor("out", [ROWS, W], dt.int32, kind="ExternalOutput")
    stages = {
        name: nc.dram_tensor(f"stage_{name}", [ROWS, cw], dt.int32, kind="Internal")
        for name, _, _, cw in SPLITS
    }

    zsrc = zcol.rearrange("(p n) one -> p (n one)", p=P)

    from contextlib import ExitStack

    es = ExitStack()
    with es:
        zv = es.enter_context(nc.sbuf_tensor("zv", [P, NQ], dt.float32))
        seeds2 = es.enter_context(nc.sbuf_tensor("seeds2", [P, NQ * CW], dt.int32))
        seedsP = es.enter_context(nc.sbuf_tensor("seedsP", [P, NQ * PW], dt.int32))
        in_sem = es.enter_context(nc.semaphore("in_sem"))
        cmp_sem = es.enter_context(nc.semaphore("cmp_sem"))
        sems = {
            name: (
                es.enter_context(nc.semaphore(f"st_{name}")),
                es.enter_context(nc.semaphore(f"dd_{name}")),
            )
            for name, _, _, _ in SPLITS
        }
        block = es.enter_context(nc.Block())

        def chain(eng, name, col0, ncols, cw, seeds_t, wait_cmp=True):
            st, dd = sems[name]
            stage = stages[name]
            if wait_cmp:
                eng.wait_ge(cmp_sem, 1)
            # stage[p*16+n, c] = seeds_t[p, (n c)]
            eng.dma_start(
                out=stage.rearrange("(p n) c -> p (n c)", p=P),
                in_=seeds_t[:, :],
            ).then_inc(st, 16)
            eng.wait_ge(st, 16)  # staging landed (same engine)
            # out[r, col0 + j*cw + c] = stage[r, c]
            nj = ncols // cw
            d2d_out = out[:, col0 : col0 + ncols].rearrange(
                "r (j c) -> r j c", c=cw
            )
            d2d_in = stage[:, :].unsqueeze(1).broadcast_to([ROWS, nj, cw])
            eng.dma_start(out=d2d_out, in_=d2d_in).then_inc(dd, 16)
            eng.wait_ge(dd, 16)  # output slice landed before kernel end

        def affine(out_ap, width):
            # out[p, n, c] = int32((zv[p, n] + 1e5) * 500), broadcast over c
            return nc.gpsimd.tensor_scalar(
                out_ap.rearrange("p (n c) -> p n c", c=width),
                zv[:, :].unsqueeze(-1).broadcast_to([P, NQ, width]),
                100000.0,
                500.0,
                alu.add,
                alu.mult,
            )

        @block.gpsimd
        def _(g):
            g.dma_start(out=zv[:, :], in_=zsrc).then_inc(in_sem, 16)
            g.wait_ge(in_sem, 16)  # true DMA completion (same engine)
            affine(seeds2[:, :], CW).then_inc(cmp_sem, 1)
            affine(seedsP[:, :], PW)
            # same engine: seedsP ready in program order
            chain(g, "gpsimd", SP_COLS + ACT_COLS, POOL_COLS, PW, seedsP, wait_cmp=False)

        @block.sync
        def _(sync):
            chain(sync, "sync", 0, SP_COLS, CW, seeds2)

        @block.scalar
        def _(scalar):
            chain(scalar, "scalar", SP_COLS, ACT_COLS, CW, seeds2)

    _nc_cache["nc"] = nc
    return nc


def kernel(z, c=None, **_unused):
    z = np.ascontiguousarray(np.asarray(z), dtype=np.float32)
    assert z.shape == (B, W), z.shape
    nc = build_nc()
    from concourse.bass_utils import run_bass_kernel_spmd

    in_maps = []
    for i in range(N_CORES):
        zc = np.ascontiguousarray(z[i * ROWS : (i + 1) * ROWS, 0:1])
        in_maps.append({"zcol": zc})
    res = run_bass_kernel_spmd(nc, in_maps, core_ids=list(range(N_CORES)))
    globals()["LAST_RESULT"] = res
    return np.concatenate([r["out"] for r in res.results], axis=0).astype(np.int32)


# revision 14
# speedup vs baseline: 1.0037x; 1.0037x over previous
"""Trainium2 Bass kernel for nn_MappingNetwork (histogram_binning).

reference: seeds = searchsorted(linspace(-1e5, 1e5, 1e8, f32), z[:, 0], 'left');
           out = broadcast(seeds[:, None], (16384, 512)).astype(int32)

The buckets are uniform with spacing 2e5/(1e8-1) ~= 0.002, so the searchsorted
index admits the closed-form affine  seed = (z + 1e5) * 500  (the spec's
sharding_hint itself suggests eliminating the bucket buffer via this closed
form).  Computed in f32 it sits within 6 indices of the bit-exact searchsorted
result — validated against an exact f64-fma emulation of the XLA-CPU linspace
over the full data range.  Relative error ~1.2e-7 against seeds of magnitude
5e7, five orders of magnitude inside the 2e-2 gate.

Per core (2048 rows, row r = p*16 + n -> zv[p, n]):
  1. the Pool engine loads the z[:, 0] shard (8KB) into [128, 16] SBUF via
     its software-DGE queue, waits on its own DMA-completion semaphore, and
     computes the seeds itself: one Pool-engine tensor_scalar producing
     width-2 seed pairs (signalled to sync/scalar immediately) and a second
     producing width-8 runs for its own slice — a single-engine
     load->compute chain with no cross-engine DMA-semaphore latency,
  2. all three DMA-capable engines INDEPENDENTLY stage their seed runs
     (16KB / 64KB) to their own internal DRAM scratch and wait on their own
     completion semaphores,
  3. three concurrent DRAM->DRAM broadcast DMAs fan each row's seed run out
     into the 4MB output shard (sync: columns 0-221, scalar: 222-441, Pool:
     442-511 — the Pool slice maximized under the software-DGE
     16K-descriptor ring limit at repeat count 7): src keeps the stride-0
     repeat dim in the middle with a contiguous innermost run (DGE-legal),
     dst walks [row, repeat, run] with every ISA dim count under 2^16.
Every engine holds the end-of-block barrier on its own DMA-completion
semaphores, so the kernel never signals done before the output is in DRAM.

Sharding: batch 16384 -> 8 cores x 2048 rows.
"""

import numpy as np

N_CORES = 8
B = 16384
W = 512
ROWS = B // N_CORES  # 2048 rows per core
P = 128
NQ = ROWS // P  # 16 queries per partition
CW = 2  # sync/scalar staged run width (seed pair)
PW = 8  # Pool staged run width
# Each engine stages its seed run DIRECTLY into the output (those columns are
# real output data), then broadcasts from them into the rest of its slice.
# (engine, direct column, broadcast columns, run width)
SPLITS = [
    ("sync", 0, 222, CW),
    ("scalar", 224, 222, CW),
    ("gpsimd", 448, 56, PW),
]

_nc_cache = {}


def build_nc():
    if "nc" in _nc_cache:
        return _nc_cache["nc"]
    import concourse.bass as bass
    import concourse.mybir as mybir

    dt = mybir.dt
    alu = mybir.AluOpType

    nc = bass.Bass(detect_race_conditions=False)
    zcol = nc.dram_tensor("zcol", [ROWS, 1], dt.float32, kind="ExternalInput")
    out = nc.dram_tensor("out", [ROWS, W], dt.int32, kind="ExternalOutput")

    zsrc = zcol.rearrange("(p n) one -> p (n one)", p=P)

    from contextlib import ExitStack

    es = ExitStack()
    with es:
        zv = es.enter_context(nc.sbuf_tensor("zv", [P, NQ], dt.float32))
        seeds2 = es.enter_context(nc.sbuf_tensor("seeds2", [P, NQ * CW], dt.int32))
        seedsP = es.enter_context(nc.sbuf_tensor("seedsP", [P, NQ * PW], dt.int32))
        in_sem = es.enter_context(nc.semaphore("in_sem"))
        cmp_sem = es.enter_context(nc.semaphore("cmp_sem"))
        sems = {
            name: (
                es.enter_context(nc.semaphore(f"st_{name}")),
                es.enter_context(nc.semaphore(f"dd_{name}")),
            )
            for name, _, _, _ in SPLITS
        }
        block = es.enter_context(nc.Block())

        def chain(eng, name, dcol, ncols_b, cw, seeds_t, wait_cmp=True):
            st, dd = sems[name]
            if wait_cmp:
                eng.wait_ge(cmp_sem, 1)
            # out[p*16+n, dcol+c] = seeds_t[p, n, c]  (direct output columns)
            eng.dma_start(
                out=out[:, dcol : dcol + cw].rearrange("(p n) c -> p n c", p=P),
                in_=seeds_t[:, :].rearrange("p (n c) -> p n c", c=cw),
            ).then_inc(st, 16)
            eng.wait_ge(st, 16)  # direct columns landed (same engine)
            # out[r, dcol+cw + j*cw + c] = out[r, dcol+c]
            nj = ncols_b // cw
            d2d_out = out[:, dcol + cw : dcol + cw + ncols_b].rearrange(
                "r (j c) -> r j c", c=cw
            )
            d2d_in = (
                out[:, dcol : dcol + cw].unsqueeze(1).broadcast_to([ROWS, nj, cw])
            )
            eng.dma_start(out=d2d_out, in_=d2d_in).then_inc(dd, 16)
            eng.wait_ge(dd, 16)  # output slice landed before kernel end

        def affine(out_ap, width):
            # out[p, n, c] = int32((zv[p, n] + 1e5) * 500), broadcast over c
            return nc.gpsimd.tensor_scalar(
                out_ap.rearrange("p (n c) -> p n c", c=width),
                zv[:, :].unsqueeze(-1).broadcast_to([P, NQ, width]),
                100000.0,
                500.0,
                alu.add,
                alu.mult,
            )

        @block.gpsimd
        def _(g):
            g.dma_start(out=zv[:, :], in_=zsrc).then_inc(in_sem, 16)
            g.wait_ge(in_sem, 16)  # true DMA completion (same engine)
            affine(seeds2[:, :], CW).then_inc(cmp_sem, 1)
            affine(seedsP[:, :], PW)
            # same engine: seedsP ready in program order
            chain(g, "gpsimd", 448, 56, PW, seedsP, wait_cmp=False)

        @block.sync
        def _(sync):
            chain(sync, "sync", 0, 222, CW, seeds2)

        @block.scalar
        def _(scalar):
            chain(scalar, "scalar", 224, 222, CW, seeds2)

    _nc_cache["nc"] = nc
    return nc


def kernel(z, c=None, **_unused):
    z = np.ascontiguousarray(np.asarray(z), dtype=np.float32)
    assert z.shape == (B, W), z.shape
    nc = build_nc()
    from concourse.bass_utils import run_bass_kernel_spmd

    in_maps = []
    for i in range(N_CORES):
        zc = np.ascontiguousarray(z[i * ROWS : (i + 1) * ROWS, 0:1])
        in_maps.append({"zcol": zc})
    res = run_bass_kernel_spmd(nc, in_maps, core_ids=list(range(N_CORES)))
    globals()["LAST_RESULT"] = res
    return np.concatenate([r["out"] for r in res.results], axis=0).astype(np.int32)
